# revision 1
# baseline (speedup 1.0000x reference)
"""Trainium2 Bass kernel for nn_BatchRNN: BatchNorm(eval) + bidirectional LSTM.

Sharding: 8 cores = 2 directions x 4 batch-groups of 16 sequences.
Backward direction handled by host-side padding-aware sequence flip (flip
commutes with per-channel BN + per-token mask), so every core runs the same
forward-scan SPMD graph with its own weights/inputs.

Device per core:
  - input projection xg^T = Wx^T @ (BN(x)*mask)^T in bf16, output laid out
    gate-transposed: xg[p, t*128 + m*16 + b] (m = 4H-chunk, b = seq)
  - 1024-step LSTM scan: 16 stationary-Wh matmuls per step produce gates
    with 4H on partitions, elementwise on [128, 96]/[128, 32] tiles,
    h written once as bf16 where the next step's matmul reads it.
"""

import sys

sys.path.insert(0, "/opt/trn_rl_repo")

import numpy as np

B, T, D, H = 64, 1024, 512, 256
H4 = 4 * H
EPS = 1e-3
P = 128
S = 16                 # sequences per core
GROUPS = B // S        # 4
KD = D // P            # 4  K-chunks for Wx
KH = H // P            # 2  K-chunks for Wh
M8 = H4 // P           # 8  M-chunks of gates
TC = 128               # time chunk
NCH = T // TC
S2 = 2 * S             # 32 = h-chunk x seq columns

_COMPILED = None
LAST_RESULT = None


def _gate_perm():
    # natural flax gate order (i, f, g, o): sigmoid-A covers i,f,g (the
    # c-path), sigmoid-B covers o (tail only, off the critical path)
    return np.arange(H4)


def _build_graph(loop_n=None):
    from concourse import bacc, bass, mybir, tile

    BF = mybir.dt.bfloat16
    F32 = mybir.dt.float32
    AF = mybir.ActivationFunctionType

    nc = bacc.Bacc("TRN2", target_bir_lowering=False, debug=False, num_devices=8)

    xT = nc.dram_tensor("xT", [D, T * S], BF, kind="ExternalInput").ap()
    msk = nc.dram_tensor("msk", [1, T * S], BF, kind="ExternalInput").ap()
    wx = nc.dram_tensor("wx", [KD, P, H4], BF, kind="ExternalInput").ap()
    wh = nc.dram_tensor("wh", [KH, P, H4], BF, kind="ExternalInput").ap()
    bn_a = nc.dram_tensor("bn_a", [P, KD], F32, kind="ExternalInput").ap()
    bn_b = nc.dram_tensor("bn_b", [P, KD], F32, kind="ExternalInput").ap()
    gb = nc.dram_tensor("gb", [P, M8], F32, kind="ExternalInput").ap()
    eye = nc.dram_tensor("eye", [P, P], BF, kind="ExternalInput").ap()
    out = nc.dram_tensor("out", [P, T * S2], BF, kind="ExternalOutput").ap()

    W = TC * S  # 2048 token-cols per chunk

    with tile.TileContext(nc) as tc:
        with (
            tc.tile_pool(name="const", bufs=1) as const,
            tc.tile_pool(name="state", bufs=1) as state,
            tc.tile_pool(name="xpool", bufs=2) as xpool,
            tc.tile_pool(name="xgpool", bufs=2) as xgpool,
            tc.tile_pool(name="hpool", bufs=2) as hpool,
            tc.tile_pool(name="spool", bufs=3) as spool,
            tc.tile_pool(name="psproj", bufs=2, space="PSUM") as psproj,
            tc.tile_pool(name="psscan", bufs=2, space="PSUM") as psscan,
        ):
            wx_sb = []
            for k in range(KD):
                tw = const.tile([P, H4], BF, tag=f"wx{k}")
                nc.sync.dma_start(tw[:], wx[k])
                wx_sb.append(tw)
            wh_sb = []
            for k in range(KH):
                tw = const.tile([P, H4], BF, tag=f"wh{k}")
                nc.sync.dma_start(tw[:], wh[k])
                wh_sb.append(tw)
            bna = const.tile([P, KD], F32, tag="bna")
            nc.sync.dma_start(bna[:], bn_a[:])
            bnb = const.tile([P, KD], F32, tag="bnb")
            nc.sync.dma_start(bnb[:], bn_b[:])
            gbt = const.tile([P, M8], F32, tag="gbt")
            nc.sync.dma_start(gbt[:], gb[:])
            eye_sb = const.tile([P, P], BF, tag="eye")
            nc.sync.dma_start(eye_sb[:], eye[:])

            cst = state.tile([P, S2], F32, tag="c")

            def body():
                nc.vector.memset(cst[:], 0.0)
                prev_h = None
                for ch in range(NCH):
                    # ---------- projection of chunk ch ----------
                    # BN affine + padding mask run on the otherwise-idle gpsimd so
                    # they don't steal ACT/DVE cycles from the scan's critical path
                    xbn = []
                    for k in range(KD):
                        xin = xpool.tile([P, W], BF, tag=f"xin{k}")
                        nc.sync.dma_start(xin[:], xT[k * P:(k + 1) * P, ch * W:(ch + 1) * W])
                        nc.gpsimd.tensor_scalar(
                            xin[:], xin[:], bna[:, k:k + 1], bnb[:, k:k + 1],
                            mybir.AluOpType.mult, mybir.AluOpType.add,
                        )
                        xbn.append(xin)
                    mrow = xpool.tile([1, W], BF, tag="mrow")
                    nc.sync.dma_start(mrow[:], msk[0:1, ch * W:(ch + 1) * W])
                    mbc = xpool.tile([P, W], BF, tag="mbc")
                    nc.gpsimd.partition_broadcast(mbc[:], mrow[0:1, :])
                    for k in range(KD):
                        nc.gpsimd.tensor_mul(xbn[k][:], xbn[k][:], mbc[:])

                    xg = xgpool.tile([P, TC * P], BF, tag="xg")
                    xg_r = xg[:].rearrange("p (t m b) -> p t m b", t=TC, m=M8, b=S)
                    for n in range(W // 512):
                        for m in range(M8):
                            ps = psproj.tile([P, 512], F32, tag="pp")
                            for k in range(KD):
                                nc.tensor.matmul(
                                    ps[:],
                                    wx_sb[k][:, m * P:(m + 1) * P],
                                    xbn[k][:, n * 512:(n + 1) * 512],
                                    start=(k == 0), stop=(k == KD - 1),
                                )
                            ps_r = ps[:].rearrange("p (t b) -> p t b", b=S)
                            # psum -> xg on DVE (gpsimd has no PSUM port), split in
                            # half to bound head-of-line blocking of scan DVE ops
                            for hlf in range(2):
                                nc.vector.tensor_scalar_add(
                                    xg_r[:, n * 32 + hlf * 16:n * 32 + (hlf + 1) * 16, m, :],
                                    ps_r[:, hlf * 16:(hlf + 1) * 16, :],
                                    gbt[:, m:m + 1],
                                )

                    # ---------- scan over chunk ch ----------
                    hb = hpool.tile([P, (TC + 1) * S2], BF, tag="hb")
                    if ch == 0:
                        nc.vector.memset(hb[:, 0:S2], 0.0)
                    else:
                        nc.vector.tensor_copy(hb[:, 0:S2], prev_h)
                    for tl in range(TC):
                        ps = psscan.tile([P, P], F32, tag="pg")
                        # xg lands in PSUM via identity matmul; issues early since
                        # it depends only on the (already-projected) xg tile.
                        nc.tensor.matmul(
                            ps[:], eye_sb[:], xg[:, tl * P:(tl + 1) * P],
                            start=True, stop=False, skip_group_check=True,
                        )
                        # k=0 matmuls first: they only need the low h-chunk, which
                        # the split h-write below makes available first.
                        for k in range(KH):
                            for m in range(M8):
                                nc.tensor.matmul(
                                    ps[:, m * S:(m + 1) * S],
                                    wh_sb[k][:, m * P:(m + 1) * P],
                                    hb[:, tl * S2 + k * S: tl * S2 + (k + 1) * S],
                                    start=False, stop=(k == KH - 1 and m == M8 - 1),
                                    skip_group_check=True,
                                )
                        # i,f,g gates (the c-path) in one sigmoid; o separately
                        # afterwards since it's only needed at the tail
                        # (g-columns pre-scaled by 2: tanh(g) == 2*sigmoid(2g)-1)
                        sg = spool.tile([P, 96], F32, tag="sg")
                        nc.scalar.activation(sg[:], ps[:, 0:96], AF.Sigmoid)
                        so = spool.tile([P, S2], F32, tag="so")
                        nc.scalar.activation(so[:], ps[:, 96:128], AF.Sigmoid)
                        # c = sf*c + si*tanh(g) with tanh(g) = 2*sigmoid(2g)-1
                        # fused via scalar_tensor_tensor:
                        #   t2 = (sg_g - 0.5)*si ; c = 2*t2 + t1
                        t1 = spool.tile([P, S2], F32, tag="t1")
                        nc.vector.tensor_mul(t1[:], sg[:, 32:64], cst[:])
                        t2 = spool.tile([P, S2], F32, tag="t2")
                        nc.vector.scalar_tensor_tensor(
                            t2[:], sg[:, 64:96], 0.5, sg[:, 0:32],
                            mybir.AluOpType.subtract, mybir.AluOpType.mult,
                        )
                        nc.vector.scalar_tensor_tensor(
                            cst[:], t2[:], 2.0, t1[:],
                            mybir.AluOpType.mult, mybir.AluOpType.add,
                        )
                        tcc = spool.tile([P, S2], F32, tag="tcc")
                        nc.scalar.activation(tcc[:], cst[:], AF.Tanh)
                        # split h write: low h-chunk first so next step's k=0
                        # matmuls can begin before the high chunk lands
                        nc.vector.tensor_mul(
                            hb[:, (tl + 1) * S2:(tl + 1) * S2 + S],
                            so[:, 0:S], tcc[:, 0:S],
                        )
                        nc.vector.tensor_mul(
                            hb[:, (tl + 1) * S2 + S:(tl + 2) * S2],
                            so[:, S:S2], tcc[:, S:S2],
                        )
                    nc.sync.dma_start(
                        out[:, ch * TC * S2:(ch + 1) * TC * S2], hb[:, S2:]
                    )
                    prev_h = hb[:, TC * S2:(TC + 1) * S2]

            if loop_n is None:
                body()
            else:
                with tc.For_i(0, loop_n, 1):
                    body()

    nc.compile()
    return nc


def _get_compiled():
    global _COMPILED
    if _COMPILED is None:
        _COMPILED = _build_graph()
    return _COMPILED


def kernel(inputs, input_paddings, bn_scale, bn_bias, bn_mean, bn_var,
           Wx_f, Wh_f, b_f, Wx_b, Wh_b, b_b):
    from concourse import mybir
    from concourse.bass_utils import run_bass_kernel_spmd

    np_bf16 = mybir.dt.np(mybir.dt.bfloat16)

    x = np.asarray(inputs, np.float32)
    pad = np.asarray(input_paddings, np.float32)
    keep = 1.0 - pad
    lengths = (T - pad.sum(axis=1)).astype(np.int64)
    idx = (np.arange(T - 1, -1, -1)[None, :] + lengths[:, None]) % T  # [B, T]
    x_flip = np.take_along_axis(x, idx[:, :, None].astype(np.int64), axis=1)

    inv = ((1.0 + np.asarray(bn_scale, np.float32))
           / np.sqrt(np.asarray(bn_var, np.float32) + EPS))
    beta = np.asarray(bn_bias, np.float32) - np.asarray(bn_mean, np.float32) * inv

    perm = _gate_perm()

    # g-gate columns scaled by 2: tanh(g) is computed as 2*sigmoid(2g)-1
    gate_scale = np.ones((H4,), np.float32)
    gate_scale[2 * H:3 * H] = 2.0  # g gate in natural (i, f, g, o) order

    def prep_w(Wx, Wh, b):
        wxp = (np.asarray(Wx, np.float32)[:, perm] * gate_scale).astype(np_bf16)
        whp = (np.asarray(Wh, np.float32)[:, perm] * gate_scale).astype(np_bf16)
        wx_t = np.stack([wxp[k * P:(k + 1) * P] for k in range(KD)])
        wh_t = np.stack([whp[k * P:(k + 1) * P] for k in range(KH)])
        gb_t = (np.asarray(b, np.float32)[perm] * gate_scale).reshape(M8, P).T.copy()
        return wx_t, wh_t, gb_t

    wx_f_t, wh_f_t, gb_f_t = prep_w(Wx_f, Wh_f, b_f)
    wx_b_t, wh_b_t, gb_b_t = prep_w(Wx_b, Wh_b, b_b)

    bn_a_t = inv.reshape(KD, P).T.copy()
    bn_b_t = beta.reshape(KD, P).T.copy()
    eye_t = np.eye(P, dtype=np.float32).astype(np_bf16)

    in_maps = []
    for core in range(8):
        fwd = core < GROUPS
        g = core % GROUPS
        sl = slice(g * S, (g + 1) * S)
        xs = (x if fwd else x_flip)[sl]                    # [S, T, D]
        xTc = np.ascontiguousarray(xs.transpose(2, 1, 0)).reshape(D, T * S)
        mskc = np.ascontiguousarray(keep[sl].T).reshape(1, T * S)
        in_maps.append(dict(
            xT=xTc.astype(np_bf16),
            msk=mskc.astype(np_bf16),
            wx=(wx_f_t if fwd else wx_b_t),
            wh=(wh_f_t if fwd else wh_b_t),
            bn_a=bn_a_t, bn_b=bn_b_t,
            gb=(gb_f_t if fwd else gb_b_t),
            eye=eye_t,
        ))

    nc = _get_compiled()
    res = run_bass_kernel_spmd(nc, in_maps, core_ids=list(range(8)))
    global LAST_RESULT
    LAST_RESULT = res

    out_full = np.zeros((B, T, 2 * H), np.float32)
    for core in range(8):
        fwd = core < GROUPS
        g = core % GROUPS
        sl = slice(g * S, (g + 1) * S)
        oc = np.asarray(res.results[core]["out"], dtype=np_bf16).astype(np.float32)
        # [p, t*32 + j*16 + b] -> [b, t, j*128+p]
        hs = oc.reshape(P, T, 2, S).transpose(3, 1, 2, 0).reshape(S, T, 2 * P)
        if fwd:
            out_full[sl, :, 0:H] = hs
        else:
            hs = np.take_along_axis(hs, idx[sl][:, :, None].astype(np.int64), axis=1)
            out_full[sl, :, H:2 * H] = hs
    return out_full

